# revision 1
# baseline (speedup 1.0000x reference)
"""GQA attention kernel for Trainium2, 8-core tensor-parallel.

Problem: B=2, T=2048, D=2048, H=32 heads, KV=8 groups, hd=64, causal + RoPE.

Sharding: 8 cores = 2 batches x 4 head-groups. Each core handles one batch
and 8 consecutive heads (= 2 KV groups): computes x @ Wq/Wk/Wv shards, RoPE,
causal attention, and a partial output through its Wo row-slice. Host sums
the 4 partials per batch.

Per-core kernel phases:
  P1: QKV projections (x streamed as lhsT, weights resident as rhs),
      RoPE in natural [t, d] layout (pair swap along free dim),
      PE-transpose Q/K into d-major [d, t] layout.
  P2: causal flash-style attention in transposed layout:
      scores_T[k, q] = K_d-major^T-slice @ Q_d-major (per 128-k-chunk),
      exp on ACT straight out of PSUM (scale=1/8 fused, no max-subtraction:
      |scores/8| <= ~6 so exp is safe in fp32),
      causal masking at 128x128 granularity (memset + one triangle mul),
      ctx_T accumulation via PE with a ones-column in V for the denominator.
  P3: out[t, :] += ctx_T-chunks^T @ Wo-slice, DMA straight from PSUM.
"""

import os
import sys

import numpy as np

for _p in ("/opt/trn_rl_repo", "/root/.axon_site/_ro/trn_rl_repo"):
    if os.path.isdir(_p) and _p not in sys.path:
        sys.path.append(_p)

from contextlib import ExitStack

import concourse.bass as bass
import concourse.tile as tile
from concourse import library_config, mybir
from concourse.bass import ds, ts
from concourse.masks import make_identity

P = 128
HD = 64            # head dim
NH = 8             # heads per core
NKV = 2            # kv groups per core
DQ = NH * HD       # 512
DKV = NKV * HD     # 128
TQ = 512           # q tile width in attention
F32 = mybir.dt.float32
SCALE = 1.0 / 8.0  # 1/sqrt(HD)

B, T_FULL, DIN_FULL, DOUT_FULL = 2, 2048, 2048, 2048
H_TOT, KV_TOT, N_CORES = 32, 8, 8
ROPE_BASE = 10000.0

# QT tile j holds heads (j, j+4) so that partitions 0:64 are always a
# group-0 head and 64:128 a group-1 head (matches packed K/V layout).
HEAD_PERM = [0, 4, 1, 5, 2, 6, 3, 7]


def _swap_pairs(ap2d, fsize):
    """View of [P, fsize] AP with adjacent free-dim pairs swapped."""
    r = ap2d.rearrange("p (a b) -> p a b", b=2)
    return r[:, :, ::-1]


def build_bass(T, DIN, DOUT, split_waits=True):
    NT = T // P
    ND = DIN // P
    NQT = T // TQ
    NDQ = DQ // P  # 4
    NDO = DOUT // TQ

    nc = bass.Bass()
    xT_d = nc.dram_tensor("xT", [DIN, T], F32, kind="ExternalInput")
    wq_d = nc.dram_tensor("wq", [DIN, DQ], F32, kind="ExternalInput")
    wkv_d = nc.dram_tensor("wkv", [DIN, 2 * DKV], F32, kind="ExternalInput")
    wo_d = nc.dram_tensor("wo", [DQ, DOUT], F32, kind="ExternalInput")
    cs_d = nc.dram_tensor("cs", [T, HD], F32, kind="ExternalInput")
    sn_d = nc.dram_tensor("sn", [T, HD], F32, kind="ExternalInput")
    mask_d = nc.dram_tensor("mask", [P, P], F32, kind="ExternalInput")
    out_d = nc.dram_tensor("out", [T, DOUT], F32, kind="ExternalOutput")

    with tile.TileContext(nc) as tc, ExitStack() as stack:
        pers = stack.enter_context(tc.tile_pool(name="pers", bufs=1))
        ps_big = stack.enter_context(tc.tile_pool(name="psbig", bufs=2, space="PSUM"))
        ps_sm = stack.enter_context(tc.tile_pool(name="pssm", bufs=2, space="PSUM"))
        ps_ctx = stack.enter_context(tc.tile_pool(name="psctx", bufs=2, space="PSUM"))
        p2pool = stack.enter_context(tc.tile_pool(name="p2pool", bufs=4))
        bcpool = stack.enter_context(tc.tile_pool(name="bcpool", bufs=3))
        drpool = stack.enter_context(tc.tile_pool(name="drpool", bufs=2, space="DRAM"))

        ident = pers.tile([P, P], F32, name="ident")
        mask_sb = pers.tile([P, P], F32, name="mask_sb")
        cs_sb = pers.tile([P, NT, HD], F32, name="cs_sb")
        sn_sb = pers.tile([P, NT, HD], F32, name="sn_sb")
        qt_tiles = [pers.tile([P, T], F32, name=f"qtt{j}") for j in range(NDQ)]
        kt_sb = pers.tile([P, T], F32, name="kt_sb")
        vp_sb = pers.tile([P, NT, 2 * (HD + 1)], F32, name="vp_sb")
        ctx_tiles = [pers.tile([P, T], F32, name=f"ctxt{j}") for j in range(NDQ)]

        make_identity(nc, ident)
        nc.sync.dma_start(out=mask_sb, in_=mask_d[:, :])
        nc.sync.dma_start(out=cs_sb, in_=cs_d.rearrange("(n p) h -> p n h", p=P))
        nc.sync.dma_start(out=sn_sb, in_=sn_d.rearrange("(n p) h -> p n h", p=P))
        nc.vector.memset(vp_sb[:, :, HD], 1.0)
        nc.vector.memset(vp_sb[:, :, 2 * HD + 1], 1.0)

        # ---------------- Phase 1: QKV + RoPE + transpose ----------------
        p1 = ExitStack()
        wpool = p1.enter_context(tc.tile_pool(name="wpool", bufs=1))
        xpool = p1.enter_context(tc.tile_pool(name="xpool", bufs=3))
        tmp = p1.enter_context(tc.tile_pool(name="tmp", bufs=2))

        wq_sb = wpool.tile([P, ND, DQ], F32, name="wq_sb")
        wkv_sb = wpool.tile([P, ND, 2 * DKV], F32, name="wkv_sb")
        wq_r = wq_d.rearrange("(n p) q -> p n q", p=P)
        wkv_r = wkv_d.rearrange("(n p) q -> p n q", p=P)
        for i in range(ND):
            nc.sync.dma_start(out=wq_sb[:, i, :], in_=wq_r[:, i, :])
            nc.sync.dma_start(out=wkv_sb[:, i, :], in_=wkv_r[:, i, :])

        xT_r = xT_d.rearrange("(n p) t -> p n t", p=P)

        for tci in range(NT):
            xc = xpool.tile([P, ND, P], F32, name="xc")
            for i in range(ND):
                nc.sync.dma_start(out=xc[:, i, :], in_=xT_r[:, i, ts(tci, P)])

            # Q projection: psum [t=128, dq=512]
            psq = ps_big.tile([P, DQ], F32, name="psq", tag="big")
            for i in range(ND):
                nc.tensor.matmul(
                    psq, lhsT=xc[:, i, :], rhs=wq_sb[:, i, :],
                    start=(i == 0), stop=(i == ND - 1),
                )
            # RoPE on Q (free-dim pair rotation), tables broadcast across heads
            csw = cs_sb[:, tci, :].unsqueeze(1).broadcast_to([P, NH, HD])
            snw = sn_sb[:, tci, :].unsqueeze(1).broadcast_to([P, NH, HD])
            t1 = tmp.tile([P, DQ], F32, name="t1")
            t2 = tmp.tile([P, DQ], F32, name="t2")
            rotq = tmp.tile([P, DQ], F32, name="rotq")
            nc.vector.tensor_mul(t1.rearrange("p (a h) -> p a h", h=HD), psq.rearrange("p (a h) -> p a h", h=HD), csw)
            nc.vector.tensor_mul(t2.rearrange("p (a h) -> p a h", h=HD), _swap_pairs(psq, DQ), snw)
            nc.vector.tensor_add(rotq, t1, t2)
            for j in range(NDQ):
                ptr = ps_ctx.tile([P, P], F32, name="ptr", tag="ctx")
                nc.tensor.transpose(ptr, rotq[:, ts(j, P)], ident)
                nc.scalar.copy(qt_tiles[j][:, ts(tci, P)], ptr)

            # K,V projection: psum [t=128, 2*DKV]
            pskv = ps_sm.tile([P, 2 * DKV], F32, name="pskv", tag="sm")
            for i in range(ND):
                nc.tensor.matmul(
                    pskv, lhsT=xc[:, i, :], rhs=wkv_sb[:, i, :],
                    start=(i == 0), stop=(i == ND - 1),
                )
            kcsw = cs_sb[:, tci, :].unsqueeze(1).broadcast_to([P, NKV, HD])
            ksnw = sn_sb[:, tci, :].unsqueeze(1).broadcast_to([P, NKV, HD])
            k1 = tmp.tile([P, DKV], F32, name="k1")
            k2 = tmp.tile([P, DKV], F32, name="k2")
            rotk = tmp.tile([P, DKV], F32, name="rotk")
            nc.vector.tensor_mul(k1.rearrange("p (a h) -> p a h", h=HD), pskv[:, 0:DKV].rearrange("p (a h) -> p a h", h=HD), kcsw)
            nc.vector.tensor_mul(k2.rearrange("p (a h) -> p a h", h=HD), _swap_pairs(pskv[:, 0:DKV], DKV), ksnw)
            nc.vector.tensor_add(rotk, k1, k2)
            ptk = ps_ctx.tile([P, P], F32, name="ptk", tag="ctx")
            nc.tensor.transpose(ptk, rotk, ident)
            nc.scalar.copy(kt_sb[:, ts(tci, P)], ptk)

            # V: no rope; copy into packed V' with ones columns
            nc.vector.tensor_copy(vp_sb[:, tci, 0:HD], pskv[:, DKV:DKV + HD])
            nc.vector.tensor_copy(vp_sb[:, tci, HD + 1:2 * HD + 1], pskv[:, DKV + HD:DKV + 2 * HD])

        p1.close()

        wopool = stack.enter_context(tc.tile_pool(name="wopool", bufs=1))
        ostpool = stack.enter_context(tc.tile_pool(name="ostpool", bufs=4))
        wo_sb = wopool.tile([P, NDQ, DOUT], F32, name="wo_sb")
        wo_r = wo_d.rearrange("(n p) q -> p n q", p=P)
        for i in range(NDQ):
            nc.sync.dma_start(out=wo_sb[:, i, :], in_=wo_r[:, i, :])

        # ---------------- Phase 2 + 3: attention + output proj ----------------
        for qi in range(NQT):
            for hl in range(NH):
                jt = hl % 4
                s = hl // 4   # kv group of this head; also partition half
                g = s
                nk = 4 * qi + 4  # number of valid k-chunks (always even)
                psc = ps_ctx.tile([HD + 1, TQ], F32, name="psc", tag="ctx")
                for c2 in range(0, nk, 2):
                    pss = ps_big.tile([P, 2 * TQ], F32, name="pss", tag="big")
                    for d in (0, 1):
                        kc = c2 + d
                        nc.tensor.matmul(
                            pss[:, ds(TQ * d, TQ)],
                            lhsT=kt_sb[HD * s:HD * s + HD, ts(kc, P)],
                            rhs=qt_tiles[jt][HD * s:HD * s + HD, ds(TQ * qi, TQ)],
                            tile_position=(HD * s, 0),
                            start=True, stop=True,
                        )
                    pt = p2pool.tile([P, 2 * TQ], F32, name="pt")
                    if c2 + 1 < 4 * qi:
                        # both chunks fully below the diagonal: one exp call
                        nc.scalar.activation(
                            pt, pss,
                            mybir.ActivationFunctionType.Exp, scale=SCALE,
                        )
                    else:
                        for d in (0, 1):
                            kc = c2 + d
                            jj = kc - 4 * qi  # >= 0 on diagonal chunks
                            base = TQ * d
                            if jj <= 0:
                                nc.scalar.activation(
                                    pt[:, ds(base, TQ)], pss[:, ds(base, TQ)],
                                    mybir.ActivationFunctionType.Exp, scale=SCALE,
                                )
                            else:
                                vs = P * jj
                                nc.gpsimd.memset(pt[:, ds(base, vs)], 0.0)
                                nc.scalar.activation(
                                    pt[:, ds(base + vs, TQ - vs)],
                                    pss[:, ds(base + vs, TQ - vs)],
                                    mybir.ActivationFunctionType.Exp, scale=SCALE,
                                )
                            if jj >= 0:
                                vs = P * jj
                                nc.vector.tensor_mul(
                                    pt[:, ds(base + vs, P)], pt[:, ds(base + vs, P)], mask_sb,
                                )
                    for d in (0, 1):
                        kc = c2 + d
                        nc.tensor.matmul(
                            psc,
                            lhsT=vp_sb[:, kc, (HD + 1) * g:(HD + 1) * g + HD + 1],
                            rhs=pt[:, ds(TQ * d, TQ)],
                            start=(kc == 0), stop=(kc == nk - 1),
                        )
                # normalize: divide by denominator (row HD of psc)
                rrow = bcpool.tile([1, TQ], F32, name="rrow")
                nc.vector.reciprocal(rrow, psc[HD:HD + 1, :])
                dr = drpool.tile([1, TQ], F32, name="dr")
                nc.sync.dma_start(out=dr, in_=rrow)
                dben = bcpool.tile([HD, TQ], F32, name="dben")
                nc.sync.dma_start(
                    out=dben,
                    in_=bass.AP(tensor=dr.tensor, offset=dr.offset, ap=[[0, HD], dr.ap[1]]),
                )
                ct, hh = hl // 2, hl % 2
                nc.vector.tensor_mul(
                    ctx_tiles[ct][HD * hh:HD * hh + HD, ds(TQ * qi, TQ)],
                    psc[0:HD, :], dben,
                )
            # output projection for this qi's t-chunks
            for tc2 in range(4 * qi, 4 * qi + 4):
                for dt in range(NDO):
                    pso = ps_big.tile([P, TQ], F32, name="pso", tag="big")
                    for c in range(NDQ):
                        nc.tensor.matmul(
                            pso,
                            lhsT=ctx_tiles[c][:, ts(tc2, P)],
                            rhs=wo_sb[:, c, ds(TQ * dt, TQ)],
                            start=(c == 0), stop=(c == NDQ - 1),
                        )
                    ost = ostpool.tile([P, TQ], F32, name="ost")
                    nc.vector.tensor_copy(ost, pso)
                    nc.sync.dma_start(out=out_d[ts(tc2, P), ds(TQ * dt, TQ)], in_=ost)

    if split_waits:
        _split_matmul_waits(nc)
    return nc


def _split_matmul_waits(nc):
    """Walrus allows only one sync-wait on a fused fp32 Matmult (S3_LW).
    Move multi-waits onto a PE NoOp inserted just before; same-engine
    program order preserves the wait semantics."""
    n = 0
    for fn in nc.m.functions:
        for blk in fn.blocks:
            new_insts = []
            for inst in blk.instructions:
                si = inst.sync_info
                if si is not None and len(si.on_wait) > 1:
                    for w in si.on_wait:
                        nop = mybir.InstNoOp(
                            name=f"WNOP-{n}",
                            engine=inst.engine,
                            sync_info=mybir.SyncInfo(on_wait=[w], on_update=[]),
                        )
                        n += 1
                        new_insts.append(nop)
                    inst.sync_info = mybir.SyncInfo(
                        on_wait=[], on_update=list(si.on_update)
                    )
                new_insts.append(inst)
            blk.instructions = new_insts
    return n


def make_tables(T):
    inv = 1.0 / (ROPE_BASE ** (np.arange(0, HD, 2, dtype=np.float32) / HD))
    ang = np.arange(T, dtype=np.float32)[:, None] * inv[None, :]  # (T, HD/2)
    c, s = np.cos(ang), np.sin(ang)
    cs = np.repeat(c, 2, axis=1).astype(np.float32)           # [c0 c0 c1 c1 ...]
    sn = np.empty((T, HD), dtype=np.float32)
    sn[:, 0::2] = -s
    sn[:, 1::2] = s
    return cs, sn


def make_mask():
    kk = np.arange(P)[:, None]
    qq = np.arange(P)[None, :]
    return (qq >= kk).astype(np.float32)


def make_core_inputs(x, Wq, Wk, Wv, Wo, T, DIN, DOUT):
    """Split full inputs into 8 per-core input maps."""
    cs, sn = make_tables(T)
    mask = make_mask()
    col_perm = np.concatenate([np.arange(HD * h, HD * h + HD) for h in HEAD_PERM])
    in_maps = []
    for c in range(N_CORES):
        b, jc = divmod(c, 4)
        hq0 = DQ * jc           # first q-column of this core's head slice
        hk0 = DKV * jc
        wq_c = np.ascontiguousarray(Wq[:, hq0:hq0 + DQ][:, col_perm])
        wkv_c = np.ascontiguousarray(
            np.concatenate([Wk[:, hk0:hk0 + DKV], Wv[:, hk0:hk0 + DKV]], axis=1)
        )
        wo_c = np.ascontiguousarray(Wo[hq0:hq0 + DQ, :])
        in_maps.append({
            "xT": np.ascontiguousarray(x[b].T),
            "wq": wq_c,
            "wkv": wkv_c,
            "wo": wo_c,
            "cs": cs,
            "sn": sn,
            "mask": mask,
        })
    return in_maps


_NC_CACHE = {}


def _get_nc(T, DIN, DOUT):
    key = (T, DIN, DOUT)
    if key not in _NC_CACHE:
        _NC_CACHE[key] = build_bass(T, DIN, DOUT)
    return _NC_CACHE[key]


def kernel(x, Wq, Wk, Wv, Wo, trace=False):
    from concourse.bass_utils import run_bass_kernel_spmd

    x = np.asarray(x, dtype=np.float32)
    Wq = np.asarray(Wq, dtype=np.float32)
    Wk = np.asarray(Wk, dtype=np.float32)
    Wv = np.asarray(Wv, dtype=np.float32)
    Wo = np.asarray(Wo, dtype=np.float32)
    T, DIN, DOUT = x.shape[1], x.shape[2], Wo.shape[1]

    nc = _get_nc(T, DIN, DOUT)
    in_maps = make_core_inputs(x, Wq, Wk, Wv, Wo, T, DIN, DOUT)
    res = run_bass_kernel_spmd(nc, in_maps, core_ids=list(range(N_CORES)), trace=trace)
    out = np.zeros((B, T, DOUT), dtype=np.float32)
    for c in range(N_CORES):
        out[c // 4] += res.results[c]["out"]
    if trace:
        return out, res
    return out

